# revision 1
# baseline (speedup 1.0000x reference)
"""Causal multi-head attention (B=4, T=2048, D=2048, H=16) on 8 Trainium2
NeuronCores via Bass/Tile, SPMD with zero collectives.

Sharding: each batch b is split over two cores by query rows using a
zigzag quarter split (core 2b: quarters Q1+Q4, core 2b+1: quarters Q2+Q3),
which balances the causal-attention triangle. Every core computes K/V
projections for its batch's full sequence (cheap redundancy that keeps the
SPMD program uniform across cores); causal masking is supplied as per-core
input data over a uniform tile pattern, so all 8 cores run the same
instruction stream.

Per-core pipeline (all matmuls in float32r — full PE rate, ~1e-4 rel err):
  0. PE-transpose x -> xT (SBUF slabs, one T/2 half at a time)
  1. K^T = Wk^T xT, Q^T = Wq^T xT (transposed layouts), V = x Wv (natural)
  2. per head: S^T tiles = K^T_chunk^T Q^T, exp on ACT (no max subtraction:
     scores are O(1) by construction), causal/pad masking by DVE multiply,
     A^T accumulated on PE with V as stationary operand, softmax denominators
     via ones-vector matmuls, normalization fused into the PSUM evacuation
  3. O rows = A^T^T Wo + bo
Outputs are the core's own (permuted) query rows; the host scatters them
back into the full [B, T, D] tensor.
"""
import numpy as np

import concourse.bacc as bacc
import concourse.mybir as mybir
from concourse.tile import TileContext
from concourse.bass_utils import run_bass_kernel_spmd

F32 = mybir.dt.float32
F32R = mybir.dt.float32r
EXP = mybir.ActivationFunctionType.Exp
MULT = mybir.AluOpType.mult

PROD_CFG = dict(B=4, T=2048, D=2048, H=16)
PIPELINE = True


def _derived(cfg):
    B, T, D, H = cfg["B"], cfg["T"], cfg["D"], cfg["H"]
    d = dict(cfg)
    d.update(
        QW=T // 4,            # quarter width (query-row shard unit)
        OWN=T // 2,           # own query rows per core
        T2=T // 2,            # xT slab half width
        DK=D // 128,          # contraction chunks
        q=T // 4 // 128,      # 128-row j-tiles per quarter
        NCH=min(512, T // 2),  # moving-N chunk for projections
        ND=min(512, D),       # phase-4 output-column slab width
        DH=128,
        N_CORES=2 * B,
    )
    return d


def _r(ap):
    return ap.bitcast(F32R)


def build_nc(cfg):
    c = _derived(cfg)
    T, D, H = c["T"], c["D"], c["H"]
    QW, OWN, T2, DK, q = c["QW"], c["OWN"], c["T2"], c["DK"], c["q"]
    NCH, ND = c["NCH"], c["ND"]
    SCALE = float(c["DH"] ** -0.5)

    nc = bacc.Bacc(
        "TRN2", target_bir_lowering=False, debug=False, num_devices=c["N_CORES"]
    )
    x = nc.dram_tensor("x", [T, D], F32R, kind="ExternalInput").ap()
    wq = nc.dram_tensor("wq", [D, D], F32R, kind="ExternalInput").ap()
    wk = nc.dram_tensor("wk", [D, D], F32R, kind="ExternalInput").ap()
    wv = nc.dram_tensor("wv", [D, D], F32R, kind="ExternalInput").ap()
    wo = nc.dram_tensor("wo", [D, D], F32R, kind="ExternalInput").ap()
    bq = nc.dram_tensor("bq", [D], F32, kind="ExternalInput").ap()
    bk = nc.dram_tensor("bk", [D], F32, kind="ExternalInput").ap()
    bv = nc.dram_tensor("bv", [D], F32, kind="ExternalInput").ap()
    bo = nc.dram_tensor("bo", [D], F32, kind="ExternalInput").ap()
    mask = nc.dram_tensor("mask", [128, 4 * q * QW], F32R, kind="ExternalInput").ap()
    ident_in = nc.dram_tensor("ident", [128, 128], F32R, kind="ExternalInput").ap()
    ones_c_in = nc.dram_tensor("ones_c", [128, 1], F32R, kind="ExternalInput").ap()
    ones_r_in = nc.dram_tensor("ones_r", [1, 128], F32R, kind="ExternalInput").ap()
    o = nc.dram_tensor("o", [OWN, D], F32, kind="ExternalOutput").ap()

    kt_d = nc.dram_tensor("kt_scratch", [D, T], F32R).ap()
    qt_d = nc.dram_tensor("qt_scratch", [D, OWN], F32R).ap()
    v_d = nc.dram_tensor("v_scratch", [T, D], F32R).ap()

    # uniform causal j-tile windows (see module docstring)
    LWIN = list(range(q)) + list(range(2 * q, 3 * q))          # L+H valid
    HONLY = list(range(q, 2 * q)) + list(range(3 * q, 4 * q))  # H valid only

    with TileContext(nc) as tc:
        with (
            tc.tile_pool(name="const", bufs=1) as pconst,
        ):
            ident = pconst.tile([128, 128], F32R, tag="ident")
            nc.sync.dma_start(out=ident[:], in_=ident_in[:])
            ones_col = pconst.tile([128, 1], F32R, tag="ones_col")
            nc.sync.dma_start(out=ones_col[:], in_=ones_c_in[:])
            ones_row = pconst.tile([1, 128], F32R, tag="ones_row")
            nc.sync.dma_start(out=ones_row[:], in_=ones_r_in[:])
            bk_sb = pconst.tile([128, DK], F32, tag="bk")
            nc.sync.dma_start(out=bk_sb[:], in_=bk.rearrange("(m p) -> p m", p=128))
            bq_sb = pconst.tile([128, DK], F32, tag="bq")
            nc.sync.dma_start(out=bq_sb[:], in_=bq.rearrange("(m p) -> p m", p=128))
            bv_sb = pconst.tile([1, D], F32R, tag="bv")
            nc.sync.dma_start(out=bv_sb[:], in_=bv[None, :].bitcast(F32R))
            bo_sb = pconst.tile([1, D], F32R, tag="bo")
            nc.sync.dma_start(out=bo_sb[:], in_=bo[None, :].bitcast(F32R))

            # ---------------- phase 0+1: xT, K^T, Q^T, V ----------------
            with (
                tc.tile_pool(name="slab", bufs=1) as pslab,
                tc.tile_pool(name="p1x", bufs=2) as p1x,
                tc.tile_pool(name="p1w", bufs=2) as p1w,
                tc.tile_pool(name="p1wv", bufs=2) as p1wv,
                tc.tile_pool(name="p1st", bufs=3) as p1st,
                tc.tile_pool(name="ps_tr", bufs=2, space="PSUM") as ps_tr,
                tc.tile_pool(name="ps_kq", bufs=2, space="PSUM") as ps_kq,
                tc.tile_pool(name="ps_v", bufs=2, space="PSUM") as ps_v,
            ):
                for hf in range(2):
                    slab = pslab.tile([128, DK * T2], F32R, tag="slab")
                    slab3 = slab[:].rearrange("p (k t) -> p k t", k=DK)
                    # transpose x rows [hf*T2, (hf+1)*T2) into slab
                    for tcn in range(T2 // 128):
                        xst = p1x.tile([128, D], F32R, tag="xst")
                        nc.sync.dma_start(
                            out=xst[:],
                            in_=x[hf * T2 + tcn * 128: hf * T2 + (tcn + 1) * 128, :],
                        )
                        for kb in range(0, DK, 4):
                            nb = min(4, DK - kb)
                            ps = ps_tr.tile([128, 512], F32R, tag="pstr")
                            for i in range(nb):
                                nc.tensor.transpose(
                                    ps[:, i * 128:(i + 1) * 128],
                                    xst[:, (kb + i) * 128:(kb + i + 1) * 128],
                                    ident[:],
                                )
                            nc.vector.tensor_copy(
                                slab3[:, kb:kb + nb, tcn * 128:(tcn + 1) * 128],
                                ps[:, : nb * 128].rearrange(
                                    "p (a b) -> p a b", a=nb
                                ),
                            )
                    # K^T (and Q^T on half 0) projections
                    projs = [(wk, bk_sb, kt_d, True)]
                    if hf == 0:
                        projs.append((wq, bq_sb, qt_d, False))
                    for w_in, b_sb, out_d, is_k in projs:
                        for m in range(DK):
                            wm = p1w.tile([128, DK * 128], F32R, tag="wm")
                            nc.sync.dma_start(
                                out=wm[:],
                                in_=w_in.rearrange("(k p) n -> p k n", p=128)[
                                    :, :, m * 128:(m + 1) * 128
                                ],
                            )
                            for jt in range(T2 // NCH):
                                ps = ps_kq.tile([128, NCH], F32, tag="pskq")
                                for k in range(DK):
                                    nc.tensor.matmul(
                                        ps[:],
                                        _r(wm[:, k * 128:(k + 1) * 128]),
                                        _r(slab[:, k * T2 + jt * NCH:
                                                k * T2 + (jt + 1) * NCH]),
                                        start=(k == 0),
                                        stop=(k == DK - 1),
                                    )
                                st = p1st.tile([128, NCH], F32R, tag="kqst")
                                nc.vector.tensor_scalar_add(
                                    st[:], ps[:], b_sb[:, m:m + 1]
                                )
                                col0 = (hf * T2 if is_k else 0) + jt * NCH
                                nc.sync.dma_start(
                                    out=out_d[m * 128:(m + 1) * 128,
                                              col0:col0 + NCH],
                                    in_=st[:],
                                )
                    # V projection (natural layout), n-chunks of 512
                    for nb_ in range(D // min(512, D)):
                        NV = min(512, D)
                        wvn = p1wv.tile([128, DK * NV], F32R, tag="wvn")
                        nc.sync.dma_start(
                            out=wvn[:],
                            in_=wv.rearrange("(k p) n -> p k n", p=128)[
                                :, :, nb_ * NV:(nb_ + 1) * NV
                            ],
                        )
                        for tcn in range(T2 // 128):
                            ps = ps_v.tile([128, NV], F32, tag="psv")
                            for k in range(DK):
                                nc.tensor.matmul(
                                    ps[:],
                                    _r(slab[:, k * T2 + tcn * 128:
                                            k * T2 + (tcn + 1) * 128]),
                                    _r(wvn[:, k * NV:(k + 1) * NV]),
                                    start=(k == 0),
                                    stop=False,
                                )
                            nc.tensor.matmul(
                                ps[:],
                                _r(ones_row[:]),
                                _r(bv_sb[:, nb_ * NV:(nb_ + 1) * NV]),
                                start=False,
                                stop=True,
                            )
                            st = p1st.tile([128, NV], F32R, tag="vst")
                            nc.scalar.copy(st[:], ps[:])
                            nc.sync.dma_start(
                                out=v_d[hf * T2 + tcn * 128:
                                        hf * T2 + (tcn + 1) * 128,
                                        nb_ * NV:(nb_ + 1) * NV],
                                in_=st[:],
                            )

            # ---------------- phase 2+3: attention per head ----------------
            with tc.tile_pool(name="aslab", bufs=1) as paslab:
              at_sb = paslab.tile([128, H * OWN], F32R, tag="aslab")
              with (
                tc.tile_pool(name="pmask", bufs=1) as pmask,
                tc.tile_pool(name="ph", bufs=2) as ph,
                tc.tile_pool(name="ppt", bufs=3) as ppt,
                tc.tile_pool(name="psm", bufs=2) as psm,
                tc.tile_pool(name="ps_s", bufs=2, space="PSUM") as ps_s,
                tc.tile_pool(name="ps_a", bufs=1, space="PSUM") as ps_a,
                tc.tile_pool(name="ps_l", bufs=1, space="PSUM") as ps_l,
            ):
                mask_sb = pmask.tile([128, 4 * q * QW], F32R, tag="mask")
                nc.sync.dma_start(out=mask_sb[:], in_=mask[:])
                NS = min(512, OWN)
                for h in range(H):
                    kt_h = ph.tile([128, T], F32R, tag="kth")
                    nc.sync.dma_start(
                        out=kt_h[:], in_=kt_d[h * 128:(h + 1) * 128, :]
                    )
                    qt_h = ph.tile([128, OWN], F32R, tag="qth")
                    nc.sync.dma_start(
                        out=qt_h[:], in_=qt_d[h * 128:(h + 1) * 128, :]
                    )
                    v_h = ph.tile([128, T], F32R, tag="vh")
                    nc.sync.dma_start(
                        out=v_h[:].rearrange("p (jb c) -> p jb c", c=128),
                        in_=v_d.rearrange("(jb p) d -> p jb d", p=128)[
                            :, :, h * 128:(h + 1) * 128
                        ],
                    )
                    psa = ps_a.tile([128, OWN], F32, tag="psa")
                    psl = ps_l.tile([1, OWN], F32, tag="psl")
                    h_own_bank = QW * 4 >= 2048
                    n_tiles = len(LWIN) + len(HONLY)

                    def consume(jb, ptv, full, pos):
                        # AV + denominator matmuls for a tile whose exp/mask
                        # chain was issued one pipeline step earlier.
                        vt = _r(v_h[:, jb * 128:(jb + 1) * 128])
                        first = pos == 0
                        # stop clears the (bank-granular) sim group flag, so in
                        # the shared-bank layout only the final H write stops
                        last_l = (pos == len(LWIN) - 1) and h_own_bank
                        last_h = pos == n_tiles - 1
                        if first and not h_own_bank:
                            nc.tensor.matmul(
                                psa[:, :OWN], vt, _r(ptv[:, :OWN]),
                                start=True, stop=False,
                            )
                            nc.tensor.matmul(
                                psl[:, :OWN], _r(ones_col[:]), _r(ptv[:, :OWN]),
                                start=True, stop=False,
                            )
                            return
                        if full:
                            nc.tensor.matmul(
                                psa[:, :QW], vt, _r(ptv[:, :QW]),
                                start=first, stop=last_l,
                            )
                            nc.tensor.matmul(
                                psa[:, QW:OWN], vt, _r(ptv[:, QW:OWN]),
                                start=first and h_own_bank, stop=last_h,
                            )
                            nc.tensor.matmul(
                                psl[:, :QW], _r(ones_col[:]), _r(ptv[:, :QW]),
                                start=first, stop=last_l,
                            )
                            nc.tensor.matmul(
                                psl[:, QW:OWN], _r(ones_col[:]),
                                _r(ptv[:, QW:OWN]),
                                start=first and h_own_bank, stop=last_h,
                            )
                        else:
                            nc.tensor.matmul(
                                psa[:, QW:OWN], vt, _r(ptv[:, :QW]),
                                start=False, stop=last_h,
                            )
                            nc.tensor.matmul(
                                psl[:, QW:OWN], _r(ones_col[:]),
                                _r(ptv[:, :QW]),
                                start=False, stop=last_h,
                            )

                    # units: full tiles singly; H-only tiles in PAIRS sharing
                    # one PSUM slot, one wide exp and one wide mask-multiply
                    # (halves ACT/DVE instruction overhead in the softmax).
                    units = [("full", (jb,)) for jb in LWIN] + [
                        ("hpair", tuple(HONLY[i:i + 2]))
                        for i in range(0, len(HONLY), 2)
                    ]
                    pos = 0
                    pending = []
                    for kind, jjs in units:
                        pss = ps_s.tile([128, OWN], F32, tag="pss")
                        pt = ppt.tile([128, OWN], F32R, tag="pt")
                        if kind == "full":
                            (jb,) = jjs
                            ns = min(NS, OWN)
                            for sc in range(OWN // ns):
                                nc.tensor.matmul(
                                    pss[:, sc * ns:(sc + 1) * ns],
                                    _r(kt_h[:, jb * 128:(jb + 1) * 128]),
                                    _r(qt_h[:, sc * ns:(sc + 1) * ns]),
                                    start=True, stop=True,
                                )
                            nc.scalar.activation(pt[:], pss[:], EXP, scale=SCALE)
                            mc = LWIN.index(jb) * QW
                            nc.vector.tensor_mul(
                                pt[:, :QW], pt[:, :QW], mask_sb[:, mc:mc + QW]
                            )
                            fresh = [(jb, pt[:], True)]
                        else:
                            for half, jb in enumerate(jjs):
                                nc.tensor.matmul(
                                    pss[:, half * QW:(half + 1) * QW],
                                    _r(kt_h[:, jb * 128:(jb + 1) * 128]),
                                    _r(qt_h[:, QW:OWN]),
                                    start=True, stop=True,
                                )
                            w = len(jjs) * QW
                            nc.scalar.activation(
                                pt[:, :w], pss[:, :w], EXP, scale=SCALE
                            )
                            mc = (2 * q + HONLY.index(jjs[0])) * QW
                            nc.vector.tensor_mul(
                                pt[:, :w], pt[:, :w], mask_sb[:, mc:mc + w]
                            )
                            fresh = [
                                (jb, pt[:, half * QW:(half + 1) * QW], False)
                                for half, jb in enumerate(jjs)
                            ]
                        if not PIPELINE:
                            pending.extend(fresh)
                            fresh = []
                        for jb_, ptv_, full_ in pending:
                            consume(jb_, ptv_, full_, pos)
                            pos += 1
                        pending = fresh
                    for jb_, ptv_, full_ in pending:
                        consume(jb_, ptv_, full_, pos)
                        pos += 1
                    # Evacuate both PSUM accumulators with fast ACT copies so
                    # the next head's matmuls aren't gated on the (slow)
                    # reciprocal / broadcast / normalize chain below.
                    l_raw = psm.tile([1, OWN], F32, tag="lraw")
                    nc.vector.tensor_copy(l_raw[:], psl[:])
                    at_raw = psm.tile([128, OWN], F32, tag="atraw")
                    nc.vector.tensor_copy(at_raw[:], psa[:])
                    l_sb = psm.tile([1, OWN], F32, tag="lsb")
                    nc.vector.reciprocal_approx_fast(l_sb[:], l_raw[:])
                    lb = psm.tile([128, OWN], F32, tag="lb")
                    nc.gpsimd.partition_broadcast(lb[:], l_sb[:], channels=128)
                    nc.vector.tensor_tensor(
                        at_sb[:, h * OWN:(h + 1) * OWN], at_raw[:], lb[:], MULT
                    )

              # ---------------- phase 4: output projection ----------------
              with (
                  tc.tile_pool(name="p4w", bufs=2) as p4w,
                  tc.tile_pool(name="p4st", bufs=2) as p4st,
                  tc.tile_pool(name="ps_o", bufs=2, space="PSUM") as ps_o,
              ):
                  for nh in range(D // ND):
                      won = p4w.tile([128, DK * ND], F32R, tag="won")
                      nc.sync.dma_start(
                          out=won[:],
                          in_=wo.rearrange("(k p) n -> p k n", p=128)[
                              :, :, nh * ND:(nh + 1) * ND
                          ],
                      )
                      for tt in range(OWN // 128):
                          pso = ps_o.tile([128, ND], F32, tag="pso")
                          for k in range(DK):
                              for sc in range(ND // min(512, ND)):
                                  NO = min(512, ND)
                                  nc.tensor.matmul(
                                      pso[:, sc * NO:(sc + 1) * NO],
                                      at_sb[:, k * OWN + tt * 128:
                                            k * OWN + (tt + 1) * 128],
                                      _r(won[:, k * ND + sc * NO:
                                             k * ND + (sc + 1) * NO]),
                                      start=(k == 0),
                                      stop=False,
                                  )
                          for sc in range(ND // min(512, ND)):
                              NO = min(512, ND)
                              nc.tensor.matmul(
                                  pso[:, sc * NO:(sc + 1) * NO],
                                  _r(ones_row[:]),
                                  _r(bo_sb[:, nh * ND + sc * NO:
                                           nh * ND + (sc + 1) * NO]),
                                  start=False,
                                  stop=True,
                              )
                          ost = p4st.tile([128, ND], F32, tag="ost")
                          nc.scalar.copy(ost[:], pso[:])
                          nc.sync.dma_start(
                              out=o[tt * 128:(tt + 1) * 128, nh * ND:(nh + 1) * ND],
                              in_=ost[:],
                          )
    nc.compile()
    return nc


def host_shard(cfg, x_full):
    """Per-core permutations, permuted x, and mask tensors.

    Returns (perms, x_ins, masks): lists indexed by core = 2*b + z.
    """
    c = _derived(cfg)
    B, T, QW, OWN, q = c["B"], c["T"], c["QW"], c["OWN"], c["q"]
    quarters = [np.arange(i * QW, (i + 1) * QW) for i in range(4)]
    LWIN = list(range(q)) + list(range(2 * q, 3 * q))
    HONLY = list(range(q, 2 * q)) + list(range(3 * q, 4 * q))
    perms, x_ins, masks = [], [], []
    for b in range(B):
        for z in range(2):
            if z == 0:
                own = [quarters[0], quarters[3]]
                rest = [quarters[1], quarters[2]]
            else:
                own = [quarters[1], quarters[2]]
                rest = [quarters[0], quarters[3]]
            perm = np.concatenate(own + rest)
            perms.append(perm)
            x_ins.append(np.ascontiguousarray(x_full[b][perm]))
            m = np.empty((128, 4 * q * QW), dtype=np.float32)
            ig_L = perm[:QW]
            ig_H = perm[QW:OWN]
            for t, jb in enumerate(LWIN):
                jg = perm[jb * 128:(jb + 1) * 128]
                m[:, t * QW:(t + 1) * QW] = (
                    jg[:, None] <= ig_L[None, :]
                ).astype(np.float32)
            for t, jb in enumerate(HONLY):
                jg = perm[jb * 128:(jb + 1) * 128]
                m[:, (2 * q + t) * QW:(2 * q + t + 1) * QW] = (
                    jg[:, None] <= ig_H[None, :]
                ).astype(np.float32)
            masks.append(m)
    return perms, x_ins, masks


def run_cores(cfg, nc, inputs, perms, x_ins, masks, trace=False, tmpdir=None):
    c = _derived(cfg)
    n = c["N_CORES"]
    f32 = np.float32
    shared = {
        "wq": np.ascontiguousarray(inputs["Wq"], f32),
        "wk": np.ascontiguousarray(inputs["Wk"], f32),
        "wv": np.ascontiguousarray(inputs["Wv"], f32),
        "wo": np.ascontiguousarray(inputs["Wo"], f32),
        "bq": np.ascontiguousarray(inputs["bq"], f32),
        "bk": np.ascontiguousarray(inputs["bk"], f32),
        "bv": np.ascontiguousarray(inputs["bv"], f32),
        "bo": np.ascontiguousarray(inputs["bo"], f32),
    }
    consts = {
        "ident": np.eye(128, dtype=f32),
        "ones_c": np.ones((128, 1), f32),
        "ones_r": np.ones((1, 128), f32),
    }
    in_maps = [
        {"x": x_ins[i], "mask": masks[i], **consts, **shared} for i in range(n)
    ]
    res = run_bass_kernel_spmd(
        nc, in_maps, list(range(n)), trace=trace, tmpdir=tmpdir
    )
    B, T, D, OWN = c["B"], c["T"], c["D"], c["OWN"]
    out = np.empty((B, T, D), dtype=np.float32)
    for b in range(B):
        for z in range(2):
            core = 2 * b + z
            out[b][perms[core][:OWN]] = res.results[core]["o"]
    return out, res


_NC_CACHE = {}


def kernel(x, Wq, bq, Wk, bk, Wv, bv, Wo, bo):
    cfg = PROD_CFG
    key = tuple(sorted(cfg.items()))
    if key not in _NC_CACHE:
        _NC_CACHE[key] = build_nc(cfg)
    nc = _NC_CACHE[key]
    x = np.asarray(x, np.float32)
    perms, x_ins, masks = host_shard(cfg, x)
    inputs = dict(Wq=Wq, bq=bq, Wk=Wk, bk=bk, Wv=Wv, bv=bv, Wo=Wo, bo=bo)
    out, _ = run_cores(cfg, nc, inputs, perms, x_ins, masks)
    return out



# revision 2
# speedup vs baseline: 1.0743x; 1.0743x over previous
"""Causal MHA (B=4, T=2048, D=2048, H=16) on 8 NeuronCores, batch x head-group
sharding (core = 2*b + g: batch b, heads [8g, 8g+8)). FLOP-ideal: each core
projects Q/K/V for its 1024 W-columns, runs causal attention for 8 heads over
the full sequence, and emits a partial O = A_loc @ Wo[rows] that the host sums
pairwise (+bo).

Per-core pipeline, all operands SBUF-resident (no DRAM scratch):
  1. K^T/Q^T = W^T xT via fp8 DoubleRow matmuls (2x PE rate, contraction
     pairs of 128-chunks); V = x Wv in bf16 (V path kept high precision).
     xT is staged from the host (no PE transposes). bk is dropped entirely
     (it only adds a per-query constant to scores -> softmax-invariant);
     bq enters via a rank-1 ones matmul; bv likewise; bo is added on host.
  2. Per head, per 1024-query chunk: S^T j-tiles with exact causal windows
     (128-granular), exp on ACT (scores are O(1): no max subtraction),
     triangular masking only on diagonal blocks; AV + softmax denominators
     accumulated with fp8 DoubleRow matmuls over j-tile PAIRS (contraction
     256). Normalization fused into PSUM evacuation (DVE+GPSIMD).
  3. O rows via fp8 DoubleRow over head pairs.
fp8 operands are pre-scaled on host (SCK/SCQ/SCW) or at evacuation (SCV/SCA);
scales fold into the exp scale and the final copies, so stored values stay in
e4m3's normal range.
"""
import numpy as np
import ml_dtypes

import concourse.bacc as bacc
import concourse.mybir as mybir
from concourse.tile import TileContext
from concourse.bass_utils import run_bass_kernel_spmd

F32 = mybir.dt.float32
BF16 = mybir.dt.bfloat16
F8 = mybir.dt.float8e4
EXP = mybir.ActivationFunctionType.Exp
COPY = mybir.ActivationFunctionType.Copy
MULT = mybir.AluOpType.mult
DR = mybir.MatmulPerfMode.DoubleRow

NP_BF16 = ml_dtypes.bfloat16
NP_F8 = ml_dtypes.float8_e4m3fn

B, T, D, H = 4, 2048, 2048, 16
HL = 8            # local heads per core
DH = 128
DK = D // 128     # 16 contraction chunks
KK = DK // 2      # 8 DoubleRow chunk-pairs
TB = T // 128     # 16 t-blocks / j-tiles
QW = 1024         # query chunk width
GW = HL * DH      # 1024 local feature columns

FP8_QK = True     # Q/K projections via fp8 DoubleRow
FP8_AV = True     # P/V in fp8, AV + denominator via DoubleRow pairs
FP8_O = True      # A/Wo in fp8, O projection via DoubleRow head pairs

SCK = 16.0        # host scale on Wk (fp8)
SCQ = 256.0       # host scale on Wq*ISQ (fp8)
SCV = 8.0         # device scale on V at evacuation (fp8)
SCA = 8.0         # device scale on A at evacuation (fp8)
SCW = 16.0        # host scale on Wo (fp8)
ISQ = float(DH) ** -0.5

DT_P = F8 if FP8_AV else BF16
NP_P = NP_F8 if FP8_AV else NP_BF16


def build_nc():
    nc = bacc.Bacc("TRN2", target_bir_lowering=False, debug=False,
                   num_devices=2 * B)
    dt_qk = F8 if FP8_QK else BF16
    xt8 = nc.dram_tensor("xt8", [128, DK * T], dt_qk, kind="ExternalInput").ap()
    xtb = nc.dram_tensor("xtb", [TB * 128, DK * DH], BF16,
                         kind="ExternalInput").ap()
    wk8 = nc.dram_tensor("wk8", [128, DK * GW], dt_qk, kind="ExternalInput").ap()
    wq8 = nc.dram_tensor("wq8", [128, DK * GW], dt_qk, kind="ExternalInput").ap()
    bq8 = nc.dram_tensor("bq8", [1, GW], dt_qk, kind="ExternalInput").ap()
    wvb = nc.dram_tensor("wvb", [128, DK * GW], BF16, kind="ExternalInput").ap()
    bvb = nc.dram_tensor("bvb", [1, GW], BF16, kind="ExternalInput").ap()
    dt_o = F8 if FP8_O else BF16
    wo8 = nc.dram_tensor("wo8", [128, HL * D], dt_o, kind="ExternalInput").ap()
    tri = nc.dram_tensor("tri", [128, 128], DT_P, kind="ExternalInput").ap()
    ones_r = nc.dram_tensor("ones_r", [1, 512], dt_qk, kind="ExternalInput").ap()
    ones_c = nc.dram_tensor("ones_c", [1, 128], BF16, kind="ExternalInput").ap()
    ones2 = nc.dram_tensor("ones2", [128, 2], DT_P, kind="ExternalInput").ap()
    o = nc.dram_tensor("o", [T, D], F32, kind="ExternalOutput").ap()

    qksc = (SCQ * SCK) if FP8_QK else 1.0
    ESC = 1.0 / qksc          # exp scale (undoes fp8 weight scaling)
    vsc = SCV if FP8_AV else 1.0
    asc = SCA if FP8_O else 1.0
    OSC = 1.0 / ((SCA * SCW) if FP8_O else 1.0)

    with TileContext(nc) as tc:
        with (
            tc.tile_pool(name="const", bufs=1) as pconst,
            tc.tile_pool(name="kt", bufs=1) as pkt,
            tc.tile_pool(name="qt", bufs=1) as pqt,
            tc.tile_pool(name="v", bufs=1) as pv,
            tc.tile_pool(name="a", bufs=1) as pa,
        ):
            tri_sb = pconst.tile([128, 128], DT_P, tag="tri")
            nc.sync.dma_start(out=tri_sb[:], in_=tri[:])
            ones_r_sb = pconst.tile([1, 512], dt_qk, tag="ones_r")
            nc.sync.dma_start(out=ones_r_sb[:], in_=ones_r[:])
            ones_c_sb = pconst.tile([1, 128], BF16, tag="ones_c")
            nc.sync.dma_start(out=ones_c_sb[:], in_=ones_c[:])
            ones2_sb = pconst.tile([128, 2], DT_P, tag="ones2")
            nc.sync.dma_start(out=ones2_sb[:], in_=ones2[:])
            bq_sb = pconst.tile([1, GW], dt_qk, tag="bq")
            nc.sync.dma_start(out=bq_sb[:], in_=bq8[:])
            bv_sb = pconst.tile([1, GW], BF16, tag="bv")
            nc.sync.dma_start(out=bv_sb[:], in_=bvb[:])

            kt_sb = pkt.tile([128, HL * T], BF16, tag="kt")
            qt_sb = pqt.tile([128, HL * T], BF16, tag="qt")
            v_sb = pv.tile([128, TB * GW], F8 if FP8_AV else BF16, tag="v")
            a_sb = pa.tile([128, HL * T], F8 if FP8_O else BF16, tag="a")
            kt3 = kt_sb[:].rearrange("p (h t) -> p h t", h=HL)
            qt3 = qt_sb[:].rearrange("p (h t) -> p h t", h=HL)
            v3 = v_sb[:].rearrange("p (j n) -> p j n", j=TB)
            a3 = a_sb[:].rearrange("p (h t) -> p h t", h=HL)

            # ---------------- phase 1: projections ----------------
            with (
                tc.tile_pool(name="xt8", bufs=1) as pxt8,
                tc.tile_pool(name="wkq", bufs=1) as pwkq,
                tc.tile_pool(name="pskq", bufs=2, space="PSUM") as pskq,
            ):
                xt8_sb = pxt8.tile([128, DK * T], dt_qk, tag="xt8")
                nc.sync.dma_start(out=xt8_sb[:], in_=xt8[:])
                wk_sb = pwkq.tile([128, DK * GW], dt_qk, tag="wk")
                nc.sync.dma_start(out=wk_sb[:], in_=wk8[:])
                wq_sb = pwkq.tile([128, DK * GW], dt_qk, tag="wq")
                nc.sync.dma_start(out=wq_sb[:], in_=wq8[:])
                if FP8_QK:
                    xt_r = xt8_sb[:].rearrange("p (kk i t) -> p kk i t", kk=KK, i=2)
                    wk_r = wk_sb[:].rearrange("p (kk i n) -> p kk i n", kk=KK, i=2)
                    wq_r = wq_sb[:].rearrange("p (kk i n) -> p kk i n", kk=KK, i=2)
                else:
                    xt_r = xt8_sb[:].rearrange("p (k t) -> p k t", k=DK)
                    wk_r = wk_sb[:].rearrange("p (k n) -> p k n", k=DK)
                    wq_r = wq_sb[:].rearrange("p (k n) -> p k n", k=DK)
                for w_r, out3, is_q in ((wk_r, kt3, False), (wq_r, qt3, True)):
                    for h in range(HL):
                        ps = pskq.tile([128, T], F32, tag="pskq")
                        if FP8_QK:
                            for kk in range(KK):
                                for c in range(4):
                                    nc.tensor.matmul(
                                        ps[:, c * 512:(c + 1) * 512],
                                        w_r[:, kk, :, h * 128:(h + 1) * 128],
                                        xt_r[:, kk, :, c * 512:(c + 1) * 512],
                                        start=(kk == 0),
                                        stop=(kk == KK - 1) and not is_q,
                                        perf_mode=DR,
                                    )
                        else:
                            for k in range(DK):
                                for c in range(4):
                                    nc.tensor.matmul(
                                        ps[:, c * 512:(c + 1) * 512],
                                        w_r[:, k, h * 128:(h + 1) * 128],
                                        xt_r[:, k, c * 512:(c + 1) * 512],
                                        start=(k == 0),
                                        stop=(k == DK - 1) and not is_q,
                                    )
                        if is_q:
                            for c in range(4):
                                nc.tensor.matmul(
                                    ps[:, c * 512:(c + 1) * 512],
                                    bq_sb[:, h * 128:(h + 1) * 128],
                                    ones_r_sb[:],
                                    start=False,
                                    stop=True,
                                )
                        nc.scalar.copy(out3[:, h, :], ps[:])

            with (
                tc.tile_pool(name="wv", bufs=1) as pwv,
                tc.tile_pool(name="xtb", bufs=3) as pxtb,
                tc.tile_pool(name="psv", bufs=2, space="PSUM") as psv,
            ):
                wv_sb = pwv.tile([128, DK * GW], BF16, tag="wv")
                nc.sync.dma_start(out=wv_sb[:], in_=wvb[:])
                wv_r = wv_sb[:].rearrange("p (k n) -> p k n", k=DK)
                for tb in range(TB):
                    xtb_t = pxtb.tile([128, DK * DH], BF16, tag="xtb")
                    nc.sync.dma_start(out=xtb_t[:],
                                      in_=xtb[tb * 128:(tb + 1) * 128, :])
                    xtb3 = xtb_t[:].rearrange("p (k c) -> p k c", k=DK)
                    ps = psv.tile([128, GW], F32, tag="psv")
                    for k in range(DK):
                        for c in range(2):
                            nc.tensor.matmul(
                                ps[:, c * 512:(c + 1) * 512],
                                xtb3[:, k, :],
                                wv_r[:, k, c * 512:(c + 1) * 512],
                                start=(k == 0),
                                stop=False,
                            )
                    for c in range(2):
                        nc.tensor.matmul(
                            ps[:, c * 512:(c + 1) * 512],
                            ones_c_sb[:],
                            bv_sb[:, c * 512:(c + 1) * 512],
                            start=False,
                            stop=True,
                        )
                    nc.scalar.activation(v3[:, tb, :], ps[:], COPY, scale=vsc)

            # ---------------- phase 2: attention ----------------
            with (
                tc.tile_pool(name="ppt", bufs=3) as ppt,
                tc.tile_pool(name="psm", bufs=2) as psm,
                tc.tile_pool(name="ps_s", bufs=2, space="PSUM") as ps_s,
                tc.tile_pool(name="ps_a", bufs=1, space="PSUM") as ps_a,
                tc.tile_pool(name="ps_l", bufs=1, space="PSUM") as ps_l,
            ):
                units = []
                for h in range(HL):
                    for qc in range(2):
                        n_jt = (qc + 1) * 8
                        wts = [max(0, jb * 128 - qc * QW) for jb in range(n_jt)]
                        n_pairs = n_jt // 2
                        last_b0 = max(p for p in range(n_pairs)
                                      if wts[2 * p] < 512)
                        for p in range(n_pairs):
                            units.append(dict(
                                h=h, qc=qc, p=p,
                                w0=wts[2 * p], w1=wts[2 * p + 1],
                                start=(p == 0), stop0=(p == last_b0),
                                stop1=(p == n_pairs - 1),
                                evac=(p == n_pairs - 1),
                            ))

                cur = {}   # live psum accumulators for one (h, qc)

                def consume(u):
                    h, qc = u["h"], u["qc"]
                    if u["p"] == 0:
                        cur["psa"] = ps_a.tile([128, QW], F32, tag="psa",
                                               name="psa")
                        cur["psl"] = ps_l.tile([1, QW], F32, tag="psl",
                                               name="psl")
                    psa, psl = cur["psa"], cur["psl"]
                    pt2, jb0 = u["pt2"], 2 * u["p"]
                    chunks = []
                    if u["w0"] < 512:
                        chunks.append((u["w0"], 512, u["stop0"]))
                        chunks.append((512, QW, u["stop1"]))
                    else:
                        chunks.append((u["w0"], QW, u["stop1"]))
                    for c0, c1, stp in chunks:
                        if FP8_AV:
                            nc.tensor.matmul(
                                psa[:, c0:c1],
                                v3[:, jb0:jb0 + 2, h * 128:(h + 1) * 128],
                                pt2[:, :, c0:c1],
                                start=u["start"], stop=stp, perf_mode=DR,
                            )
                            nc.tensor.matmul(
                                psl[:, c0:c1], ones2_sb[:], pt2[:, :, c0:c1],
                                start=u["start"], stop=stp, perf_mode=DR,
                            )
                        else:
                            for i in range(2):
                                w = u["w0"] if i == 0 else u["w1"]
                                d0 = max(c0, w)
                                if d0 >= c1:
                                    continue
                                nc.tensor.matmul(
                                    psa[:, d0:c1],
                                    v3[:, jb0 + i, h * 128:(h + 1) * 128],
                                    pt2[:, i, d0:c1],
                                    start=u["start"] and i == 0,
                                    stop=stp and i == 1, skip_group_check=True,
                                )
                                nc.tensor.matmul(
                                    psl[:, d0:c1], ones2_sb[:, 0:1],
                                    pt2[:, i, d0:c1],
                                    start=u["start"] and i == 0,
                                    stop=stp and i == 1, skip_group_check=True,
                                )
                    if u["evac"]:
                        l_raw = psm.tile([1, QW], F32, tag="lraw")
                        nc.vector.tensor_copy(l_raw[:], psl[:])
                        at_raw = psm.tile([128, QW], F32, tag="atraw")
                        nc.vector.tensor_copy(at_raw[:], psa[:])
                        if vsc != asc:
                            nc.vector.tensor_scalar_mul(
                                l_raw[:], l_raw[:], float(vsc / asc))
                        rl = psm.tile([1, QW], F32, tag="rl")
                        nc.vector.reciprocal_approx_fast(rl[:], l_raw[:])
                        lb = psm.tile([128, QW], F32, tag="lb")
                        nc.gpsimd.partition_broadcast(lb[:], rl[:], channels=128)
                        nc.vector.tensor_tensor(
                            a3[:, h, qc * QW:(qc + 1) * QW],
                            at_raw[:], lb[:], MULT)

                pending = None
                for u in units:
                    h, qc = u["h"], u["qc"]
                    pt2_t = ppt.tile([128, 2 * QW], DT_P, tag="pt2")
                    pt2 = pt2_t[:].rearrange("p (i t) -> p i t", i=2)
                    u["pt2"] = pt2
                    for i in range(2):
                        w = u["w0"] if i == 0 else u["w1"]
                        jb = 2 * u["p"] + i
                        pss = ps_s.tile([128, QW], F32, tag="pss")
                        for c0, c1 in ((w, 512), (512, QW)):
                            c0 = max(c0, w)
                            if c0 >= c1:
                                continue
                            nc.tensor.matmul(
                                pss[:, c0:c1],
                                kt3[:, h, jb * 128:(jb + 1) * 128],
                                qt3[:, h, qc * QW + c0:qc * QW + c1],
                                start=True, stop=True,
                            )
                        nc.scalar.activation(pt2[:, i, w:QW], pss[:, w:QW],
                                             EXP, scale=ESC)
                        if jb * 128 >= qc * QW:  # diagonal tile
                            nc.vector.tensor_mul(
                                pt2[:, i, w:w + 128], pt2[:, i, w:w + 128],
                                tri_sb[:])
                        if FP8_AV and i == 1 and u["w1"] > u["w0"]:
                            nc.vector.memset(pt2[:, 1, u["w0"]:u["w1"]], 0)
                    if pending is not None:
                        consume(pending)
                    pending = u
                consume(pending)

            # ---------------- phase 3: output projection ----------------
            with (
                tc.tile_pool(name="wo", bufs=1) as pwo,
                tc.tile_pool(name="ost", bufs=2) as post,
                tc.tile_pool(name="ps_o", bufs=2, space="PSUM") as ps_o,
            ):
                wo_sb = pwo.tile([128, HL * D], dt_o, tag="wo")
                nc.sync.dma_start(out=wo_sb[:], in_=wo8[:])
                wo3 = wo_sb[:].rearrange("p (h n) -> p h n", h=HL)
                for tb in range(TB):
                    pso = ps_o.tile([128, D], F32, tag="pso")
                    if FP8_O:
                        for hp in range(HL // 2):
                            for c in range(4):
                                nc.tensor.matmul(
                                    pso[:, c * 512:(c + 1) * 512],
                                    a3[:, 2 * hp:2 * hp + 2,
                                       tb * 128:(tb + 1) * 128],
                                    wo3[:, 2 * hp:2 * hp + 2,
                                        c * 512:(c + 1) * 512],
                                    start=(hp == 0), stop=(hp == HL // 2 - 1),
                                    perf_mode=DR,
                                )
                    else:
                        for h in range(HL):
                            for c in range(4):
                                nc.tensor.matmul(
                                    pso[:, c * 512:(c + 1) * 512],
                                    a3[:, h, tb * 128:(tb + 1) * 128],
                                    wo3[:, h, c * 512:(c + 1) * 512],
                                    start=(h == 0), stop=(h == HL - 1),
                                )
                    ost = post.tile([128, D], F32, tag="ost")
                    nc.scalar.activation(ost[:], pso[:], COPY, scale=OSC)
                    nc.sync.dma_start(out=o[tb * 128:(tb + 1) * 128, :],
                                      in_=ost[:])
    nc.compile()
    return nc


def host_prepare(inputs):
    """Build the 8 per-core input maps from full inputs."""
    x = np.asarray(inputs["x"], np.float32)
    np_qk = NP_F8 if FP8_QK else NP_BF16
    np_o = NP_F8 if FP8_O else NP_BF16
    sck = SCK if FP8_QK else 1.0
    scq = SCQ if FP8_QK else 1.0
    scw = SCW if FP8_O else 1.0
    consts = {
        "tri": np.triu(np.ones((128, 128), np.float32)).astype(NP_P),
        "ones_r": np.ones((1, 512), np_qk),
        "ones_c": np.ones((1, 128), NP_BF16),
        "ones2": np.ones((128, 2), NP_P),
    }
    gmaps = []
    for g in range(2):
        cs = slice(g * GW, (g + 1) * GW)
        wk = np.asarray(inputs["Wk"], np.float32)[:, cs] * sck
        wq = np.asarray(inputs["Wq"], np.float32)[:, cs] * (ISQ * scq)
        bq = np.asarray(inputs["bq"], np.float32)[cs] * (ISQ * scq)
        wv = np.asarray(inputs["Wv"], np.float32)[:, cs]
        bv = np.asarray(inputs["bv"], np.float32)[cs]
        wo = np.asarray(inputs["Wo"], np.float32)[cs, :] * scw
        if FP8_QK:
            # [p, kk, i, n] with contraction row (2kk+i)*128+p
            wk_l = wk.reshape(KK, 2, 128, GW).transpose(2, 0, 1, 3)
            wq_l = wq.reshape(KK, 2, 128, GW).transpose(2, 0, 1, 3)
        else:
            wk_l = wk.reshape(DK, 128, GW).transpose(1, 0, 2)
            wq_l = wq.reshape(DK, 128, GW).transpose(1, 0, 2)
        gmaps.append({
            "wk8": np.ascontiguousarray(wk_l.reshape(128, DK * GW)).astype(np_qk),
            "wq8": np.ascontiguousarray(wq_l.reshape(128, DK * GW)).astype(np_qk),
            "bq8": bq.reshape(1, GW).astype(np_qk),
            "wvb": np.ascontiguousarray(
                wv.reshape(DK, 128, GW).transpose(1, 0, 2)
                .reshape(128, DK * GW)).astype(NP_BF16),
            "bvb": bv.reshape(1, GW).astype(NP_BF16),
            "wo8": np.ascontiguousarray(
                wo.reshape(HL, 128, D).transpose(1, 0, 2)
                .reshape(128, HL * D)).astype(np_o),
        })
    in_maps = []
    for b in range(B):
        xT = np.ascontiguousarray(x[b].T)          # [D, T]
        if FP8_QK:
            xt8_l = xT.reshape(KK, 2, 128, T).transpose(2, 0, 1, 3)
        else:
            xt8_l = xT.reshape(DK, 128, T).transpose(1, 0, 2)
        xt8 = np.ascontiguousarray(xt8_l.reshape(128, DK * T)).astype(
            NP_F8 if FP8_QK else NP_BF16)
        # xtb[tb, p, k, c] = xT[k*128+p, tb*128+c]
        xtb = np.ascontiguousarray(
            xT.reshape(DK, 128, TB, 128).transpose(2, 1, 0, 3)
            .reshape(TB * 128, DK * DH)).astype(NP_BF16)
        for g in range(2):
            in_maps.append({"xt8": xt8, "xtb": xtb, **gmaps[g], **consts})
    return in_maps


_NC_CACHE = {}


def run_cores(nc, in_maps, trace=False, tmpdir=None):
    return run_bass_kernel_spmd(nc, in_maps, list(range(2 * B)), trace=trace,
                                tmpdir=tmpdir)


def kernel(x, Wq, bq, Wk, bk, Wv, bv, Wo, bo):
    if "nc" not in _NC_CACHE:
        _NC_CACHE["nc"] = build_nc()
    nc = _NC_CACHE["nc"]
    inputs = dict(x=x, Wq=Wq, bq=bq, Wk=Wk, Wv=Wv, bv=bv, Wo=Wo)
    in_maps = host_prepare(inputs)
    res = run_bass_kernel_spmd(nc, in_maps, list(range(2 * B)))
    bo = np.asarray(bo, np.float32)
    out = np.empty((B, T, D), np.float32)
    for b in range(B):
        out[b] = res.results[2 * b]["o"] + res.results[2 * b + 1]["o"] + bo
    return out
